# revision 11
# baseline (speedup 1.0000x reference)
"""Trainium2 Bass kernel: segment_sum of edge features into nodes (GNN aggregation).

out[n, :] = sum of edges[e, :] over edges with receivers[e] == n, for
n in [0, 100000), edges [1000000, 64] fp32 — distributed over 8 NeuronCores.
Cores are value-sharded by receiver range (12500 nodes each, disjoint), so no
cross-core reduction is needed; the host concatenates the shards.

Device algorithm (degree-slotted static-ones matmul):
  - Host sorts each core's edges by receiver and packs them into "node-rows"
    of 4/2/1 slots (three regions by degree remainder, minimizing padding);
    a chunk = 128 slots = one TensorEngine matmul.
  - The stationary operand is a STATIC block-ones matrix (e.g. [128, 32] with
    ones[s, j] = 1 iff s//4 == j): out row j = sum of row j's slots. There is
    no per-chunk weight generation at all (no one-hot; VectorEngine is idle).
  - Edge fp32 values ride as fp16 hi + fp16 lo halves in one matmul: the
    output access pattern wraps both 64-column halves onto the same PSUM
    columns, and PSUM's per-element has_written accumulate folds hi+lo in
    hardware (end-to-end error ~2e-7 relative).
  - Column tiling (tile_position=(0, 32b)) packs 4 chunks per 128-partition
    PSUM block; 7 blocks share one PSUM bank; a single ScalarEngine copy
    flushes the bank and a contiguous DMA writes the rows out. Input DMAs run
    on the Sync-engine HWDGE ring, output DMAs on the Scalar-engine ring so
    the two streams don't serialize on one FIFO.
  - Host folds the ~3 rows per node with np.add.reduceat (region S4) and
    vectorized adds (S2/S1), then fixes up any capacity-spilled edges.
"""

import os

import numpy as np

N_EDGES = 1_000_000
N_NODES = 100_000
N_FEAT = 64
N_CORES = 8
NODES_PER_CORE = N_NODES // N_CORES  # 12500
BLK_W = 7

N4_BLOCKS = 245  # rows of 4 slots: cap 31360 (mean ~30500)
N2_BLOCKS = 28   # rows of 2 slots: cap 3584 (mean ~3125)
N1_BLOCKS = 28   # rows of 1 slot:  cap 3584 (mean ~3125)
N_BLOCKS = N4_BLOCKS + N2_BLOCKS + N1_BLOCKS  # 301
R4_CAP = N4_BLOCKS * 128
R2_CAP = N2_BLOCKS * 128
R1_CAP = N1_BLOCKS * 128
C4 = N4_BLOCKS * 4  # chunks in S4 region
C2 = N2_BLOCKS * 2
C1 = N1_BLOCKS * 1
C_CHUNKS = C4 + C2 + C1  # 1064

_NC_CACHE = None
LAST_RESULT = None


def _build_nc():
    global _NC_CACHE
    if _NC_CACHE is not None:
        return _NC_CACHE

    import concourse.bass as bass
    import concourse.tile as tile
    from concourse import bacc, mybir

    F16 = mybir.dt.float16
    F32 = mybir.dt.float32

    nc = bacc.Bacc("TRN2", target_bir_lowering=False)
    tokens = nc.dram_tensor("tokens", [128, C_CHUNKS, 128], F16, kind="ExternalInput")
    ones4 = nc.dram_tensor("ones4", [128, 32], F16, kind="ExternalInput")
    ones2 = nc.dram_tensor("ones2", [128, 64], F16, kind="ExternalInput")
    ones1 = nc.dram_tensor("ones1", [128, 128], F16, kind="ExternalInput")
    out = nc.dram_tensor("out", [128, N_BLOCKS, 64], F32, kind="ExternalOutput")

    with tile.TileContext(nc) as tc:
        with (
            tc.tile_pool(name="const", bufs=1) as const,
            tc.tile_pool(name="tok", bufs=8) as tokp,
            tc.tile_pool(name="ps", bufs=4, space="PSUM") as psp,
            tc.tile_pool(name="stage", bufs=3) as stp,
        ):
            ones4_t = const.tile([128, 32], F16)
            nc.sync.dma_start(ones4_t[:], ones4[:])
            ones2_t = const.tile([128, 64], F16)
            nc.sync.dma_start(ones2_t[:], ones2[:])
            ones1_t = const.tile([128, 128], F16)
            nc.sync.dma_start(ones1_t[:], ones1[:])

            def emit_mm(ps, blk, b, cols, ones_t, tok, lc):
                # out rows [cols*b, cols*(b+1)) of block blk, wrapped hi|lo
                pslice = ps[cols * b : cols * (b + 1),
                            blk * 64 : (blk + 1) * 64]
                o = bass.AP(pslice.tensor, pslice.offset,
                            [list(pslice.ap[0]), [0, 2], [1, 64]])
                nc.tensor.matmul(
                    out=o, lhsT=ones_t[:], rhs=tok[:, lc, :],
                    start=True, stop=True, tile_position=(0, cols * b))

            # region descriptors: (n_blocks, chunks_per_block, ones tile,
            #                      out-rows per chunk, chunk_base, block_base)
            # Emit the small S1/S2 regions first: their small token DMAs fill
            # the pipeline ramp; the uniform S4 stream then runs saturated.
            regions = [
                (N1_BLOCKS, 1, ones1_t, 128, C4 + C2, N4_BLOCKS + N2_BLOCKS),
                (N2_BLOCKS, 2, ones2_t, 64, C4, N4_BLOCKS),
                (N4_BLOCKS, 4, ones4_t, 32, 0, 0),
            ]
            for n_blocks, cpb, ones_t, cols, cbase, bbase in regions:
                group_sizes = [BLK_W] * (n_blocks // BLK_W)
                assert sum(group_sizes) == n_blocks
                b0 = 0
                for gw in group_sizes:
                    c0 = cbase + b0 * cpb
                    tok = tokp.tile([128, BLK_W * 4, 128], F16, tag="tok")
                    nc.sync.dma_start(
                        tok[:, 0 : gw * cpb, :], tokens[:, c0 : c0 + gw * cpb, :])
                    ps = psp.tile([128, BLK_W * 64], F32, tag="ps")
                    for blk in range(gw):
                        for b in range(cpb):
                            emit_mm(ps, blk, b, cols, ones_t, tok,
                                    blk * cpb + b)
                    stage = stp.tile([128, BLK_W * 64], F32, tag="stage")
                    nc.scalar.copy(stage[:, 0 : gw * 64], ps[:, 0 : gw * 64])
                    nc.scalar.dma_start(
                        out[:, bbase + b0 : bbase + b0 + gw, :],
                        stage[:, 0 : gw * 64])
                    b0 += gw
    nc.compile()
    _NC_CACHE = nc
    return nc


def _numpy_segment_sum(edges, receivers):
    out = np.zeros((N_NODES, N_FEAT), np.float32)
    r = np.asarray(receivers).astype(np.int64)
    ok = (r >= 0) & (r < N_NODES)
    np.add.at(out, r[ok], np.asarray(edges, np.float32)[ok])
    return out


def kernel(edges, nodes, receivers):
    global LAST_RESULT

    edges = np.ascontiguousarray(edges, dtype=np.float32)
    n_nodes = nodes.shape[0]
    r = np.asarray(receivers).astype(np.int64)
    if (
        edges.shape != (N_EDGES, N_FEAT)
        or n_nodes != N_NODES
        or r.shape != (N_EDGES,)
        or os.environ.get("KERNEL_FORCE_NUMPY")
    ):
        return _numpy_segment_sum(edges, receivers)

    order = np.argsort(r, kind="stable")
    r_s = r[order]
    bounds = np.searchsorted(r_s, NODES_PER_CORE * np.arange(N_CORES + 1))

    hi_all = edges.astype(np.float16)
    lo_all = (edges - hi_all.astype(np.float32)).astype(np.float16)

    ar = np.arange(128)
    ones4 = (ar[None, :] // 1 == 0).astype(np.float16)  # placeholder, fixed below
    ones4 = np.zeros((128, 32), np.float16)
    ones4[ar, ar // 4] = 1.0
    ones2 = np.zeros((128, 64), np.float16)
    ones2[ar, ar // 2] = 1.0
    ones1 = np.zeros((128, 128), np.float16)
    ones1[ar, ar] = 1.0

    in_maps = []
    spill_ids = []
    meta = []
    for i in range(N_CORES):
        lo_b, hi_b = bounds[i], bounds[i + 1]
        idx = order[lo_b:hi_b]
        rr = r_s[lo_b:hi_b] - NODES_PER_CORE * i

        d = np.bincount(rr, minlength=NODES_PER_CORE)
        rem = d & 3
        rows4_n = (d >> 2) + (rem == 3)
        rows2_n = (rem == 2).astype(np.int64)
        rows1_n = (rem == 1).astype(np.int64)
        if (
            rows4_n.sum() > R4_CAP
            or rows2_n.sum() > R2_CAP
            or rows1_n.sum() > R1_CAP
        ):
            cut = min(
                int(np.searchsorted(np.cumsum(rows4_n), R4_CAP, side="right")),
                int(np.searchsorted(np.cumsum(rows2_n), R2_CAP, side="right")),
                int(np.searchsorted(np.cumsum(rows1_n), R1_CAP, side="right")),
            )
            sp = rr >= cut
            spill_ids.append(idx[sp])
            idx, rr = idx[~sp], rr[~sp]
            d = np.bincount(rr, minlength=NODES_PER_CORE)
            rem = d & 3
            rows4_n = (d >> 2) + (rem == 3)
            rows2_n = (rem == 2).astype(np.int64)
            rows1_n = (rem == 1).astype(np.int64)

        def excl_cumsum(a):
            s = np.zeros_like(a)
            np.cumsum(a[:-1], out=s[1:])
            return s

        rs4 = excl_cumsum(rows4_n)
        rs2 = excl_cumsum(rows2_n)
        rs1 = excl_cumsum(rows1_n)
        node_first = excl_cumsum(d)

        rank = np.arange(len(rr)) - node_first[rr]
        e_rem = rem[rr]
        n_s4_edges = np.where(e_rem == 3, d[rr], (d[rr] >> 2) << 2)
        m4 = rank < n_s4_edges
        m2 = (~m4) & (e_rem == 2)
        m1 = (~m4) & (e_rem == 1)

        tokens = np.zeros((128, C_CHUNKS, 128), np.float16)

        row4 = rs4[rr[m4]] + (rank[m4] >> 2)
        lc = row4 >> 5
        p = (row4 & 31) * 4 + (rank[m4] & 3)
        tokens[p, lc, 0:64] = hi_all[idx[m4]]
        tokens[p, lc, 64:128] = lo_all[idx[m4]]

        row2 = rs2[rr[m2]]
        slot2 = rank[m2] - n_s4_edges[m2]
        lc = C4 + (row2 >> 6)
        p = (row2 & 63) * 2 + slot2
        tokens[p, lc, 0:64] = hi_all[idx[m2]]
        tokens[p, lc, 64:128] = lo_all[idx[m2]]

        row1 = rs1[rr[m1]]
        lc = C4 + C2 + (row1 >> 7)
        p = row1 & 127
        tokens[p, lc, 0:64] = hi_all[idx[m1]]
        tokens[p, lc, 64:128] = lo_all[idx[m1]]

        in_maps.append(
            {"tokens": tokens, "ones4": ones4, "ones2": ones2, "ones1": ones1}
        )
        meta.append((rows4_n, rs4, rows2_n, rs2, rows1_n, rs1))

    from concourse.bass_utils import run_bass_kernel_spmd

    nc = _build_nc()
    res = run_bass_kernel_spmd(nc, in_maps, core_ids=list(range(N_CORES)))
    LAST_RESULT = res

    full = np.zeros((N_NODES, N_FEAT), np.float32)
    for i in range(N_CORES):
        dev = res.results[i]["out"]  # [128, N_BLOCKS, 64]
        rows4_n, rs4, rows2_n, rs2, rows1_n, rs1 = meta[i]
        block = full[i * NODES_PER_CORE : (i + 1) * NODES_PER_CORE]

        arr4 = dev[:, 0:N4_BLOCKS, :].transpose(1, 0, 2).reshape(R4_CAP, 64)
        nz = rows4_n > 0
        if nz.any():
            block[nz] = np.add.reduceat(arr4, rs4[nz])

        arr2 = (
            dev[:, N4_BLOCKS : N4_BLOCKS + N2_BLOCKS, :]
            .transpose(1, 0, 2)
            .reshape(R2_CAP, 64)
        )
        m2n = rows2_n > 0
        if m2n.any():
            block[m2n] += arr2[rs2[m2n]]

        arr1 = (
            dev[:, N4_BLOCKS + N2_BLOCKS :, :].transpose(1, 0, 2).reshape(R1_CAP, 64)
        )
        m1n = rows1_n > 0
        if m1n.any():
            block[m1n] += arr1[rs1[m1n]]

    if spill_ids:
        sp = np.concatenate(spill_ids)
        np.add.at(full, r[sp], edges[sp])

    return full


# revision 12
# speedup vs baseline: 1.0269x; 1.0269x over previous
"""Trainium2 Bass kernel: segment_sum of edge features into nodes (GNN aggregation).

out[n, :] = sum of edges[e, :] over edges with receivers[e] == n, for
n in [0, 100000), edges [1000000, 64] fp32 — distributed over 8 NeuronCores.
Cores are value-sharded by receiver range (12500 nodes each, disjoint), so no
cross-core reduction is needed; the host concatenates the shards.

Device algorithm (degree-slotted static-ones matmul):
  - Host sorts each core's edges by receiver and packs them into "node-rows"
    of 4/2/1 slots (three regions by degree remainder, minimizing padding);
    a chunk = 128 slots = one TensorEngine matmul.
  - The stationary operand is a STATIC block-ones matrix (e.g. [128, 32] with
    ones[s, j] = 1 iff s//4 == j): out row j = sum of row j's slots. There is
    no per-chunk weight generation at all (no one-hot; VectorEngine is idle).
  - Edge fp32 values ride as fp16 hi + fp16 lo halves in one matmul: the
    output access pattern wraps both 64-column halves onto the same PSUM
    columns, and PSUM's per-element has_written accumulate folds hi+lo in
    hardware (end-to-end error ~2e-7 relative).
  - Column tiling (tile_position=(0, 32b)) packs 4 chunks per 128-partition
    PSUM block; 7 blocks share one PSUM bank; a single ScalarEngine copy
    flushes the bank and a contiguous DMA writes the rows out. Input DMAs run
    on the Sync-engine HWDGE ring, output DMAs on the Scalar-engine ring so
    the two streams don't serialize on one FIFO.
  - Host folds the ~3 rows per node with np.add.reduceat (region S4) and
    vectorized adds (S2/S1), then fixes up any capacity-spilled edges.
"""

import os

import numpy as np

N_EDGES = 1_000_000
N_NODES = 100_000
N_FEAT = 64
N_CORES = 8
NODES_PER_CORE = N_NODES // N_CORES  # 12500
BLK_W = 7

N4_BLOCKS = 245  # rows of 4 slots: cap 31360 (mean ~30500)
N2_BLOCKS = 28   # rows of 2 slots: cap 3584 (mean ~3125)
N1_BLOCKS = 28   # rows of 1 slot:  cap 3584 (mean ~3125)
N_BLOCKS = N4_BLOCKS + N2_BLOCKS + N1_BLOCKS  # 301
R4_CAP = N4_BLOCKS * 128
R2_CAP = N2_BLOCKS * 128
R1_CAP = N1_BLOCKS * 128
C4 = N4_BLOCKS * 4  # chunks in S4 region
C2 = N2_BLOCKS * 2
C1 = N1_BLOCKS * 1
C_CHUNKS = C4 + C2 + C1  # 1064

_NC_CACHE = None
LAST_RESULT = None


def _build_nc():
    global _NC_CACHE
    if _NC_CACHE is not None:
        return _NC_CACHE

    import concourse.bass as bass
    import concourse.tile as tile
    from concourse import bacc, mybir

    F16 = mybir.dt.float16
    F32 = mybir.dt.float32

    nc = bacc.Bacc("TRN2", target_bir_lowering=False)
    tokens = nc.dram_tensor("tokens", [128, C_CHUNKS, 128], F16, kind="ExternalInput")
    ones4 = nc.dram_tensor("ones4", [128, 32], F16, kind="ExternalInput")
    ones2 = nc.dram_tensor("ones2", [128, 64], F16, kind="ExternalInput")
    ones1 = nc.dram_tensor("ones1", [128, 128], F16, kind="ExternalInput")
    out = nc.dram_tensor("out", [128, N_BLOCKS, 64], F32, kind="ExternalOutput")

    with tile.TileContext(nc) as tc:
        with (
            tc.tile_pool(name="const", bufs=1) as const,
            tc.tile_pool(name="tok", bufs=6) as tokp,
            tc.tile_pool(name="ps", bufs=3, space="PSUM") as psp,
            tc.tile_pool(name="stage", bufs=3) as stp,
        ):
            ones4_t = const.tile([128, 32], F16)
            nc.sync.dma_start(ones4_t[:], ones4[:])
            ones2_t = const.tile([128, 64], F16)
            nc.sync.dma_start(ones2_t[:], ones2[:])
            ones1_t = const.tile([128, 128], F16)
            nc.sync.dma_start(ones1_t[:], ones1[:])

            def emit_mm(ps, blk, b, cols, ones_t, tok, lc):
                # out rows [cols*b, cols*(b+1)) of block blk, wrapped hi|lo
                pslice = ps[cols * b : cols * (b + 1),
                            blk * 64 : (blk + 1) * 64]
                o = bass.AP(pslice.tensor, pslice.offset,
                            [list(pslice.ap[0]), [0, 2], [1, 64]])
                nc.tensor.matmul(
                    out=o, lhsT=ones_t[:], rhs=tok[:, lc, :],
                    start=True, stop=True, tile_position=(0, cols * b))

            # region descriptors: (n_blocks, chunks_per_block, ones tile,
            #                      out-rows per chunk, chunk_base, block_base)
            # Emit the small S1/S2 regions first: their small token DMAs fill
            # the pipeline ramp; the uniform S4 stream then runs saturated.
            regions = [
                (N1_BLOCKS, 1, ones1_t, 128, C4 + C2, N4_BLOCKS + N2_BLOCKS),
                (N2_BLOCKS, 2, ones2_t, 64, C4, N4_BLOCKS),
                (N4_BLOCKS, 4, ones4_t, 32, 0, 0),
            ]
            for n_blocks, cpb, ones_t, cols, cbase, bbase in regions:
                group_sizes = [BLK_W] * (n_blocks // BLK_W)
                assert sum(group_sizes) == n_blocks
                b0 = 0
                for gw in group_sizes:
                    c0 = cbase + b0 * cpb
                    tok = tokp.tile([128, BLK_W * 4, 128], F16, tag="tok")
                    nc.sync.dma_start(
                        tok[:, 0 : gw * cpb, :], tokens[:, c0 : c0 + gw * cpb, :])
                    ps = psp.tile([128, BLK_W * 64], F32, tag="ps")
                    for blk in range(gw):
                        for b in range(cpb):
                            emit_mm(ps, blk, b, cols, ones_t, tok,
                                    blk * cpb + b)
                    stage = stp.tile([128, BLK_W * 64], F32, tag="stage")
                    nc.scalar.copy(stage[:, 0 : gw * 64], ps[:, 0 : gw * 64])
                    nc.scalar.dma_start(
                        out[:, bbase + b0 : bbase + b0 + gw, :],
                        stage[:, 0 : gw * 64])
                    b0 += gw
    nc.compile()
    _NC_CACHE = nc
    return nc


def _numpy_segment_sum(edges, receivers):
    out = np.zeros((N_NODES, N_FEAT), np.float32)
    r = np.asarray(receivers).astype(np.int64)
    ok = (r >= 0) & (r < N_NODES)
    np.add.at(out, r[ok], np.asarray(edges, np.float32)[ok])
    return out


def kernel(edges, nodes, receivers):
    global LAST_RESULT

    edges = np.ascontiguousarray(edges, dtype=np.float32)
    n_nodes = nodes.shape[0]
    r = np.asarray(receivers).astype(np.int64)
    if (
        edges.shape != (N_EDGES, N_FEAT)
        or n_nodes != N_NODES
        or r.shape != (N_EDGES,)
        or os.environ.get("KERNEL_FORCE_NUMPY")
    ):
        return _numpy_segment_sum(edges, receivers)

    order = np.argsort(r, kind="stable")
    r_s = r[order]
    bounds = np.searchsorted(r_s, NODES_PER_CORE * np.arange(N_CORES + 1))

    hi_all = edges.astype(np.float16)
    lo_all = (edges - hi_all.astype(np.float32)).astype(np.float16)

    ar = np.arange(128)
    ones4 = (ar[None, :] // 1 == 0).astype(np.float16)  # placeholder, fixed below
    ones4 = np.zeros((128, 32), np.float16)
    ones4[ar, ar // 4] = 1.0
    ones2 = np.zeros((128, 64), np.float16)
    ones2[ar, ar // 2] = 1.0
    ones1 = np.zeros((128, 128), np.float16)
    ones1[ar, ar] = 1.0

    in_maps = []
    spill_ids = []
    meta = []
    for i in range(N_CORES):
        lo_b, hi_b = bounds[i], bounds[i + 1]
        idx = order[lo_b:hi_b]
        rr = r_s[lo_b:hi_b] - NODES_PER_CORE * i

        d = np.bincount(rr, minlength=NODES_PER_CORE)
        rem = d & 3
        rows4_n = (d >> 2) + (rem == 3)
        rows2_n = (rem == 2).astype(np.int64)
        rows1_n = (rem == 1).astype(np.int64)
        if (
            rows4_n.sum() > R4_CAP
            or rows2_n.sum() > R2_CAP
            or rows1_n.sum() > R1_CAP
        ):
            cut = min(
                int(np.searchsorted(np.cumsum(rows4_n), R4_CAP, side="right")),
                int(np.searchsorted(np.cumsum(rows2_n), R2_CAP, side="right")),
                int(np.searchsorted(np.cumsum(rows1_n), R1_CAP, side="right")),
            )
            sp = rr >= cut
            spill_ids.append(idx[sp])
            idx, rr = idx[~sp], rr[~sp]
            d = np.bincount(rr, minlength=NODES_PER_CORE)
            rem = d & 3
            rows4_n = (d >> 2) + (rem == 3)
            rows2_n = (rem == 2).astype(np.int64)
            rows1_n = (rem == 1).astype(np.int64)

        def excl_cumsum(a):
            s = np.zeros_like(a)
            np.cumsum(a[:-1], out=s[1:])
            return s

        rs4 = excl_cumsum(rows4_n)
        rs2 = excl_cumsum(rows2_n)
        rs1 = excl_cumsum(rows1_n)
        node_first = excl_cumsum(d)

        rank = np.arange(len(rr)) - node_first[rr]
        e_rem = rem[rr]
        n_s4_edges = np.where(e_rem == 3, d[rr], (d[rr] >> 2) << 2)
        m4 = rank < n_s4_edges
        m2 = (~m4) & (e_rem == 2)
        m1 = (~m4) & (e_rem == 1)

        tokens = np.zeros((128, C_CHUNKS, 128), np.float16)

        row4 = rs4[rr[m4]] + (rank[m4] >> 2)
        lc = row4 >> 5
        p = (row4 & 31) * 4 + (rank[m4] & 3)
        tokens[p, lc, 0:64] = hi_all[idx[m4]]
        tokens[p, lc, 64:128] = lo_all[idx[m4]]

        row2 = rs2[rr[m2]]
        slot2 = rank[m2] - n_s4_edges[m2]
        lc = C4 + (row2 >> 6)
        p = (row2 & 63) * 2 + slot2
        tokens[p, lc, 0:64] = hi_all[idx[m2]]
        tokens[p, lc, 64:128] = lo_all[idx[m2]]

        row1 = rs1[rr[m1]]
        lc = C4 + C2 + (row1 >> 7)
        p = row1 & 127
        tokens[p, lc, 0:64] = hi_all[idx[m1]]
        tokens[p, lc, 64:128] = lo_all[idx[m1]]

        in_maps.append(
            {"tokens": tokens, "ones4": ones4, "ones2": ones2, "ones1": ones1}
        )
        meta.append((rows4_n, rs4, rows2_n, rs2, rows1_n, rs1))

    from concourse.bass_utils import run_bass_kernel_spmd

    nc = _build_nc()
    res = run_bass_kernel_spmd(nc, in_maps, core_ids=list(range(N_CORES)))
    LAST_RESULT = res

    full = np.zeros((N_NODES, N_FEAT), np.float32)
    for i in range(N_CORES):
        dev = res.results[i]["out"]  # [128, N_BLOCKS, 64]
        rows4_n, rs4, rows2_n, rs2, rows1_n, rs1 = meta[i]
        block = full[i * NODES_PER_CORE : (i + 1) * NODES_PER_CORE]

        arr4 = dev[:, 0:N4_BLOCKS, :].transpose(1, 0, 2).reshape(R4_CAP, 64)
        nz = rows4_n > 0
        if nz.any():
            block[nz] = np.add.reduceat(arr4, rs4[nz])

        arr2 = (
            dev[:, N4_BLOCKS : N4_BLOCKS + N2_BLOCKS, :]
            .transpose(1, 0, 2)
            .reshape(R2_CAP, 64)
        )
        m2n = rows2_n > 0
        if m2n.any():
            block[m2n] += arr2[rs2[m2n]]

        arr1 = (
            dev[:, N4_BLOCKS + N2_BLOCKS :, :].transpose(1, 0, 2).reshape(R1_CAP, 64)
        )
        m1n = rows1_n > 0
        if m1n.any():
            block[m1n] += arr1[rs1[m1n]]

    if spill_ids:
        sp = np.concatenate(spill_ids)
        np.add.at(full, r[sp], edges[sp])

    return full


# revision 13
# speedup vs baseline: 1.0506x; 1.0231x over previous
"""Trainium2 Bass kernel: segment_sum of edge features into nodes (GNN aggregation).

out[n, :] = sum of edges[e, :] over edges with receivers[e] == n, for
n in [0, 100000), edges [1000000, 64] fp32 — distributed over 8 NeuronCores.
Cores are value-sharded by receiver range (12500 nodes each, disjoint), so no
cross-core reduction is needed; the host concatenates the shards.

Device algorithm (degree-slotted static-ones matmul):
  - Host sorts each core's edges by receiver and packs them into "node-rows"
    of 4/2/1 slots (three regions by degree remainder, minimizing padding);
    a chunk = 128 slots = one TensorEngine matmul.
  - The stationary operand is a STATIC block-ones matrix (e.g. [128, 32] with
    ones[s, j] = 1 iff s//4 == j): out row j = sum of row j's slots. There is
    no per-chunk weight generation at all (no one-hot; VectorEngine is idle).
  - Edge fp32 values ride as fp16 hi + fp16 lo halves in one matmul: the
    output access pattern wraps both 64-column halves onto the same PSUM
    columns, and PSUM's per-element has_written accumulate folds hi+lo in
    hardware (end-to-end error ~2e-7 relative).
  - Column tiling (tile_position=(0, 32b)) packs 4 chunks per 128-partition
    PSUM block; 7 blocks share one PSUM bank; a single ScalarEngine copy
    flushes the bank and a contiguous DMA writes the rows out. Input DMAs run
    on the Sync-engine HWDGE ring, output DMAs on the Scalar-engine ring so
    the two streams don't serialize on one FIFO.
  - Host folds the ~3 rows per node with np.add.reduceat (region S4) and
    vectorized adds (S2/S1), then fixes up any capacity-spilled edges.
"""

import os

import numpy as np

N_EDGES = 1_000_000
N_NODES = 100_000
N_FEAT = 64
N_CORES = 8
NODES_PER_CORE = N_NODES // N_CORES  # 12500
BLK_W = 7

N4_BLOCKS = 242  # rows of 4 slots: cap 30976 (mean ~30500, +3.7 sigma)
N2_BLOCKS = 26   # rows of 2 slots: cap 3328 (mean ~3125, +4 sigma)
N1_BLOCKS = 26   # rows of 1 slot:  cap 3328 (mean ~3125, +4 sigma)
N_BLOCKS = N4_BLOCKS + N2_BLOCKS + N1_BLOCKS  # 301
R4_CAP = N4_BLOCKS * 128
R2_CAP = N2_BLOCKS * 128
R1_CAP = N1_BLOCKS * 128
C4 = N4_BLOCKS * 4  # chunks in S4 region
C2 = N2_BLOCKS * 2
C1 = N1_BLOCKS * 1
C_CHUNKS = C4 + C2 + C1  # 1064

_NC_CACHE = None
LAST_RESULT = None


def _build_nc():
    global _NC_CACHE
    if _NC_CACHE is not None:
        return _NC_CACHE

    import concourse.bass as bass
    import concourse.tile as tile
    from concourse import bacc, mybir

    F16 = mybir.dt.float16
    F32 = mybir.dt.float32

    nc = bacc.Bacc("TRN2", target_bir_lowering=False)
    tokens = nc.dram_tensor("tokens", [128, C_CHUNKS, 128], F16, kind="ExternalInput")
    ones4 = nc.dram_tensor("ones4", [128, 32], F16, kind="ExternalInput")
    ones2 = nc.dram_tensor("ones2", [128, 64], F16, kind="ExternalInput")
    ones1 = nc.dram_tensor("ones1", [128, 128], F16, kind="ExternalInput")
    out = nc.dram_tensor("out", [128, N_BLOCKS, 64], F32, kind="ExternalOutput")

    with tile.TileContext(nc) as tc:
        with (
            tc.tile_pool(name="const", bufs=1) as const,
            tc.tile_pool(name="tok", bufs=6) as tokp,
            tc.tile_pool(name="ps", bufs=3, space="PSUM") as psp,
            tc.tile_pool(name="stage", bufs=3) as stp,
        ):
            ones4_t = const.tile([128, 32], F16)
            nc.sync.dma_start(ones4_t[:], ones4[:])
            ones2_t = const.tile([128, 64], F16)
            nc.sync.dma_start(ones2_t[:], ones2[:])
            ones1_t = const.tile([128, 128], F16)
            nc.sync.dma_start(ones1_t[:], ones1[:])

            def emit_mm(ps, blk, b, cols, ones_t, tok, lc):
                # out rows [cols*b, cols*(b+1)) of block blk, wrapped hi|lo
                pslice = ps[cols * b : cols * (b + 1),
                            blk * 64 : (blk + 1) * 64]
                o = bass.AP(pslice.tensor, pslice.offset,
                            [list(pslice.ap[0]), [0, 2], [1, 64]])
                nc.tensor.matmul(
                    out=o, lhsT=ones_t[:], rhs=tok[:, lc, :],
                    start=True, stop=True, tile_position=(0, cols * b))

            # region descriptors: (chunks_per_block, ones tile,
            #                      out-rows per chunk, chunk_base, block_base)
            regions = [
                (1, ones1_t, 128, C4 + C2, N4_BLOCKS + N2_BLOCKS),  # S1
                (2, ones2_t, 64, C4, N4_BLOCKS),                    # S2
                (4, ones4_t, 32, 0, 0),                             # S4
            ]

            def split_groups(n_blocks, tail):
                full = (n_blocks - sum(tail)) // BLK_W
                sizes = [BLK_W] * full + tail
                assert sum(sizes) == n_blocks
                b0, out_l = 0, []
                for gw in sizes:
                    out_l.append((b0, gw))
                    b0 += gw
                return out_l

            g1 = [(0, b0, gw) for b0, gw in split_groups(N1_BLOCKS, [5])]
            g2 = [(1, b0, gw) for b0, gw in split_groups(N2_BLOCKS, [5])]
            g4 = [(2, b0, gw) for b0, gw in split_groups(N4_BLOCKS, [2, 1, 1])]
            # Interleave: tiny S1/S2 groups prime the pipeline and then pad
            # bubbles between the big S4 groups; ramp-down tail ends the run.
            emit = [g1[0], g2[0]]
            small = g1[1:] + g2[1:]
            rest = []
            for k, grp in enumerate(g4):
                rest.append(grp)
                if k % 3 == 2 and small:
                    rest.append(small.pop(0))
            emit += rest + small

            for ridx, b0, gw in emit:
                cpb, ones_t, cols, cbase, bbase = regions[ridx]
                c0 = cbase + b0 * cpb
                tok = tokp.tile([128, BLK_W * 4, 128], F16, tag="tok")
                nc.sync.dma_start(
                    tok[:, 0 : gw * cpb, :], tokens[:, c0 : c0 + gw * cpb, :])
                ps = psp.tile([128, BLK_W * 64], F32, tag="ps")
                for blk in range(gw):
                    for b in range(cpb):
                        emit_mm(ps, blk, b, cols, ones_t, tok, blk * cpb + b)
                stage = stp.tile([128, BLK_W * 64], F32, tag="stage")
                nc.scalar.copy(stage[:, 0 : gw * 64], ps[:, 0 : gw * 64])
                nc.scalar.dma_start(
                    out[:, bbase + b0 : bbase + b0 + gw, :],
                    stage[:, 0 : gw * 64])
    nc.compile()
    _NC_CACHE = nc
    return nc


def _numpy_segment_sum(edges, receivers):
    out = np.zeros((N_NODES, N_FEAT), np.float32)
    r = np.asarray(receivers).astype(np.int64)
    ok = (r >= 0) & (r < N_NODES)
    np.add.at(out, r[ok], np.asarray(edges, np.float32)[ok])
    return out


def kernel(edges, nodes, receivers):
    global LAST_RESULT

    edges = np.ascontiguousarray(edges, dtype=np.float32)
    n_nodes = nodes.shape[0]
    r = np.asarray(receivers).astype(np.int64)
    if (
        edges.shape != (N_EDGES, N_FEAT)
        or n_nodes != N_NODES
        or r.shape != (N_EDGES,)
        or os.environ.get("KERNEL_FORCE_NUMPY")
    ):
        return _numpy_segment_sum(edges, receivers)

    order = np.argsort(r, kind="stable")
    r_s = r[order]
    bounds = np.searchsorted(r_s, NODES_PER_CORE * np.arange(N_CORES + 1))

    hi_all = edges.astype(np.float16)
    lo_all = (edges - hi_all.astype(np.float32)).astype(np.float16)

    ar = np.arange(128)
    ones4 = (ar[None, :] // 1 == 0).astype(np.float16)  # placeholder, fixed below
    ones4 = np.zeros((128, 32), np.float16)
    ones4[ar, ar // 4] = 1.0
    ones2 = np.zeros((128, 64), np.float16)
    ones2[ar, ar // 2] = 1.0
    ones1 = np.zeros((128, 128), np.float16)
    ones1[ar, ar] = 1.0

    in_maps = []
    spill_ids = []
    meta = []
    for i in range(N_CORES):
        lo_b, hi_b = bounds[i], bounds[i + 1]
        idx = order[lo_b:hi_b]
        rr = r_s[lo_b:hi_b] - NODES_PER_CORE * i

        d = np.bincount(rr, minlength=NODES_PER_CORE)
        rem = d & 3
        rows4_n = (d >> 2) + (rem == 3)
        rows2_n = (rem == 2).astype(np.int64)
        rows1_n = (rem == 1).astype(np.int64)
        if (
            rows4_n.sum() > R4_CAP
            or rows2_n.sum() > R2_CAP
            or rows1_n.sum() > R1_CAP
        ):
            cut = min(
                int(np.searchsorted(np.cumsum(rows4_n), R4_CAP, side="right")),
                int(np.searchsorted(np.cumsum(rows2_n), R2_CAP, side="right")),
                int(np.searchsorted(np.cumsum(rows1_n), R1_CAP, side="right")),
            )
            sp = rr >= cut
            spill_ids.append(idx[sp])
            idx, rr = idx[~sp], rr[~sp]
            d = np.bincount(rr, minlength=NODES_PER_CORE)
            rem = d & 3
            rows4_n = (d >> 2) + (rem == 3)
            rows2_n = (rem == 2).astype(np.int64)
            rows1_n = (rem == 1).astype(np.int64)

        def excl_cumsum(a):
            s = np.zeros_like(a)
            np.cumsum(a[:-1], out=s[1:])
            return s

        rs4 = excl_cumsum(rows4_n)
        rs2 = excl_cumsum(rows2_n)
        rs1 = excl_cumsum(rows1_n)
        node_first = excl_cumsum(d)

        rank = np.arange(len(rr)) - node_first[rr]
        e_rem = rem[rr]
        n_s4_edges = np.where(e_rem == 3, d[rr], (d[rr] >> 2) << 2)
        m4 = rank < n_s4_edges
        m2 = (~m4) & (e_rem == 2)
        m1 = (~m4) & (e_rem == 1)

        tokens = np.zeros((128, C_CHUNKS, 128), np.float16)

        row4 = rs4[rr[m4]] + (rank[m4] >> 2)
        lc = row4 >> 5
        p = (row4 & 31) * 4 + (rank[m4] & 3)
        tokens[p, lc, 0:64] = hi_all[idx[m4]]
        tokens[p, lc, 64:128] = lo_all[idx[m4]]

        row2 = rs2[rr[m2]]
        slot2 = rank[m2] - n_s4_edges[m2]
        lc = C4 + (row2 >> 6)
        p = (row2 & 63) * 2 + slot2
        tokens[p, lc, 0:64] = hi_all[idx[m2]]
        tokens[p, lc, 64:128] = lo_all[idx[m2]]

        row1 = rs1[rr[m1]]
        lc = C4 + C2 + (row1 >> 7)
        p = row1 & 127
        tokens[p, lc, 0:64] = hi_all[idx[m1]]
        tokens[p, lc, 64:128] = lo_all[idx[m1]]

        in_maps.append(
            {"tokens": tokens, "ones4": ones4, "ones2": ones2, "ones1": ones1}
        )
        meta.append((rows4_n, rs4, rows2_n, rs2, rows1_n, rs1))

    from concourse.bass_utils import run_bass_kernel_spmd

    nc = _build_nc()
    res = run_bass_kernel_spmd(nc, in_maps, core_ids=list(range(N_CORES)))
    LAST_RESULT = res

    full = np.zeros((N_NODES, N_FEAT), np.float32)
    for i in range(N_CORES):
        dev = res.results[i]["out"]  # [128, N_BLOCKS, 64]
        rows4_n, rs4, rows2_n, rs2, rows1_n, rs1 = meta[i]
        block = full[i * NODES_PER_CORE : (i + 1) * NODES_PER_CORE]

        arr4 = dev[:, 0:N4_BLOCKS, :].transpose(1, 0, 2).reshape(R4_CAP, 64)
        nz = rows4_n > 0
        if nz.any():
            block[nz] = np.add.reduceat(arr4, rs4[nz])

        arr2 = (
            dev[:, N4_BLOCKS : N4_BLOCKS + N2_BLOCKS, :]
            .transpose(1, 0, 2)
            .reshape(R2_CAP, 64)
        )
        m2n = rows2_n > 0
        if m2n.any():
            block[m2n] += arr2[rs2[m2n]]

        arr1 = (
            dev[:, N4_BLOCKS + N2_BLOCKS :, :].transpose(1, 0, 2).reshape(R1_CAP, 64)
        )
        m1n = rows1_n > 0
        if m1n.any():
            block[m1n] += arr1[rs1[m1n]]

    if spill_ids:
        sp = np.concatenate(spill_ids)
        np.add.at(full, r[sp], edges[sp])

    return full


# revision 14
# speedup vs baseline: 1.1021x; 1.0490x over previous
"""Trainium2 Bass kernel: segment_sum of edge features into nodes (GNN aggregation).

out[n, :] = sum of edges[e, :] over edges with receivers[e] == n, for
n in [0, 100000), edges [1000000, 64] fp32 — distributed over 8 NeuronCores.
Cores are value-sharded by receiver range (12500 nodes each, disjoint), so no
cross-core reduction is needed; the host concatenates the shards.

Device algorithm (degree-slotted static-ones matmul, fused):
  - Host sorts each core's edges by receiver and packs them into "node-rows"
    of 4/2/1 slots (three regions by degree remainder, minimizing padding);
    a chunk = 128 slots.
  - The stationary operand is a STATIC block-ones matrix (e.g. [128, 32] with
    ones[s, j] = 1 iff s//4 == j): out row j = sum of row j's slots. No
    per-chunk weight generation at all (no one-hot; VectorEngine is idle).
  - Edge fp32 values ride as fp16 hi + fp16 lo halves: the matmul output
    access pattern wraps both 64-column halves onto the same PSUM columns and
    PSUM's per-element has_written accumulate folds hi+lo in hardware
    (end-to-end error ~2e-7 relative).
  - One matmul processes up to 4 chunks (rhs free dim 512, 4D wrapped out
    AP), cutting the PE instruction stream ~4x; column tiling
    (tile_position=(0, 32b)) packs 4 chunk-columns per 128-partition PSUM
    block, 8 blocks fill one 2KB PSUM bank exactly; one ScalarEngine copy
    flushes the bank. Input DMAs ride the Sync-engine HWDGE ring, output DMAs
    the Scalar-engine ring so the streams don't serialize on one FIFO.
  - Small S1/S2 groups are interleaved into the S4 stream to prime the DMA
    pipeline and pad bubbles; group sizes ramp down at the tail.
  - Host folds the ~3 rows per node with np.add.reduceat (S4) and vectorized
    adds (S2/S1), then fixes up any capacity-spilled edges.
"""

import os

import numpy as np

N_EDGES = 1_000_000
N_NODES = 100_000
N_FEAT = 64
N_CORES = 8
NODES_PER_CORE = N_NODES // N_CORES  # 12500

# group sizes (in 128-row blocks); shared by host packing and device schedule
G4 = [8] * 29 + [4, 2, 1, 1]  # 240 blocks: rows cap 30720 (mean ~30500)
G2 = [8] * 3  # 24 blocks: cap 3072 (mean ~3125; small spills possible)
G1 = [8] * 3  # 24 blocks: cap 3072
N4_BLOCKS = sum(G4)
N2_BLOCKS = sum(G2)
N1_BLOCKS = sum(G1)
N_BLOCKS = N4_BLOCKS + N2_BLOCKS + N1_BLOCKS  # 288
R4_CAP = N4_BLOCKS * 128
R2_CAP = N2_BLOCKS * 128
R1_CAP = N1_BLOCKS * 128
C4 = N4_BLOCKS * 4
C2 = N2_BLOCKS * 2
C1 = N1_BLOCKS * 1
C_CHUNKS = C4 + C2 + C1  # 1032

_NC_CACHE = None
LAST_RESULT = None


def _region_layout(groups, cpb):
    """Per-group (block_start, chunk_start) offsets within a region."""
    bs, cs, out = 0, 0, []
    for gw in groups:
        out.append((bs, cs))
        bs += gw
        cs += gw * cpb
    return out


_L4 = _region_layout(G4, 4)
_L2 = _region_layout(G2, 2)
_L1 = _region_layout(G1, 1)


def _row_maps():
    """Vectorized row-id -> (chunk index, column j) maps per region.

    Within a group of gw blocks, chunks are laid out b-major (lc = b*gw +
    blkin) so one matmul's rhs spans up to 4 consecutive chunks of the same
    column-group b across blocks.
    """
    maps = {}
    for name, groups, layout, cpb, cbase in (
        ("s4", G4, _L4, 4, 0),
        ("s2", G2, _L2, 2, C4),
        ("s1", G1, _L1, 1, C4 + C2),
    ):
        cap = sum(groups) * 128
        rows = np.arange(cap)
        block = rows >> 7
        gstarts = np.array([b for b, _ in layout])
        g = np.searchsorted(gstarts, block, side="right") - 1
        blkin = block - gstarts[g]
        gw = np.array(groups)[g]
        cstart = cbase + np.array([c for _, c in layout])[g]
        rows_per_col = 128 // cpb  # 32 / 64 / 128
        b = (rows & 127) // rows_per_col
        j = (rows & 127) % rows_per_col
        lc = cstart + b * gw + blkin
        maps[name] = (lc.astype(np.int64), j.astype(np.int64))
    return maps


_ROW_MAPS = _row_maps()


def _build_nc():
    global _NC_CACHE
    if _NC_CACHE is not None:
        return _NC_CACHE

    import concourse.bass as bass
    import concourse.tile as tile
    from concourse import bacc, mybir

    F16 = mybir.dt.float16
    F32 = mybir.dt.float32

    nc = bacc.Bacc("TRN2", target_bir_lowering=False)
    tokens = nc.dram_tensor("tokens", [128, C_CHUNKS, 128], F16, kind="ExternalInput")
    ones4 = nc.dram_tensor("ones4", [128, 32], F16, kind="ExternalInput")
    ones2 = nc.dram_tensor("ones2", [128, 64], F16, kind="ExternalInput")
    ones1 = nc.dram_tensor("ones1", [128, 128], F16, kind="ExternalInput")
    out = nc.dram_tensor("out", [128, N_BLOCKS, 64], F32, kind="ExternalOutput")

    with tile.TileContext(nc) as tc:
        with (
            tc.tile_pool(name="const", bufs=1) as const,
            tc.tile_pool(name="tok", bufs=6) as tokp,
            tc.tile_pool(name="ps", bufs=3, space="PSUM") as psp,
            tc.tile_pool(name="stage", bufs=3) as stp,
        ):
            ones4_t = const.tile([128, 32], F16)
            nc.sync.dma_start(ones4_t[:], ones4[:])
            ones2_t = const.tile([128, 64], F16)
            nc.sync.dma_start(ones2_t[:], ones2[:])
            ones1_t = const.tile([128, 128], F16)
            nc.sync.dma_start(ones1_t[:], ones1[:])

            # (groups, layout, cpb, ones tile, out-rows/chunk, cbase, bbase)
            regs = [
                (G1, _L1, 1, ones1_t, 128, C4 + C2, N4_BLOCKS + N2_BLOCKS),
                (G2, _L2, 2, ones2_t, 64, C4, N4_BLOCKS),
                (G4, _L4, 4, ones4_t, 32, 0, 0),
            ]
            emit = [(0, 0), (1, 0)]
            small = [(0, k) for k in range(1, len(G1))] + [
                (1, k) for k in range(1, len(G2))
            ]
            for k in range(len(G4)):
                emit.append((2, k))
                if k % 3 == 2 and small:
                    emit.append(small.pop(0))
            emit += small

            for ridx, gidx in emit:
                groups, layout, cpb, ones_t, cols, cbase, bbase = regs[ridx]
                gw = groups[gidx]
                b0, cs = layout[gidx]
                c0 = cbase + cs
                nchunks = gw * cpb
                tok = tokp.tile([128, 8 * 4, 128], F16, tag="tok")
                nc.sync.dma_start(
                    tok[:, 0:nchunks, :], tokens[:, c0 : c0 + nchunks, :])
                ps = psp.tile([128, 8 * 64], F32, tag="ps")
                for b in range(cpb):
                    for blk0 in range(0, gw, 4):
                        w = min(4, gw - blk0)
                        pslice = ps[cols * b : cols * (b + 1),
                                    blk0 * 64 : (blk0 + w) * 64]
                        o = bass.AP(
                            pslice.tensor, pslice.offset,
                            [list(pslice.ap[0]), [64, w], [0, 2], [1, 64]])
                        nc.tensor.matmul(
                            out=o, lhsT=ones_t[:],
                            rhs=tok[:, b * gw + blk0 : b * gw + blk0 + w, :],
                            start=True, stop=True,
                            tile_position=(0, cols * b))
                stage = stp.tile([128, 8 * 64], F32, tag="stage")
                nc.scalar.copy(stage[:, 0 : gw * 64], ps[:, 0 : gw * 64])
                nc.scalar.dma_start(
                    out[:, bbase + b0 : bbase + b0 + gw, :],
                    stage[:, 0 : gw * 64])
    nc.compile()
    _NC_CACHE = nc
    return nc


def _numpy_segment_sum(edges, receivers):
    out = np.zeros((N_NODES, N_FEAT), np.float32)
    r = np.asarray(receivers).astype(np.int64)
    ok = (r >= 0) & (r < N_NODES)
    np.add.at(out, r[ok], np.asarray(edges, np.float32)[ok])
    return out


def kernel(edges, nodes, receivers):
    global LAST_RESULT

    edges = np.ascontiguousarray(edges, dtype=np.float32)
    n_nodes = nodes.shape[0]
    r = np.asarray(receivers).astype(np.int64)
    if (
        edges.shape != (N_EDGES, N_FEAT)
        or n_nodes != N_NODES
        or r.shape != (N_EDGES,)
        or os.environ.get("KERNEL_FORCE_NUMPY")
    ):
        return _numpy_segment_sum(edges, receivers)

    order = np.argsort(r, kind="stable")
    r_s = r[order]
    bounds = np.searchsorted(r_s, NODES_PER_CORE * np.arange(N_CORES + 1))

    hi_all = edges.astype(np.float16)
    lo_all = (edges - hi_all.astype(np.float32)).astype(np.float16)

    ar = np.arange(128)
    ones4 = np.zeros((128, 32), np.float16)
    ones4[ar, ar // 4] = 1.0
    ones2 = np.zeros((128, 64), np.float16)
    ones2[ar, ar // 2] = 1.0
    ones1 = np.zeros((128, 128), np.float16)
    ones1[ar, ar] = 1.0

    lc4_map, j4_map = _ROW_MAPS["s4"]
    lc2_map, j2_map = _ROW_MAPS["s2"]
    lc1_map, j1_map = _ROW_MAPS["s1"]

    in_maps = []
    spill_ids = []
    meta = []
    for i in range(N_CORES):
        lo_b, hi_b = bounds[i], bounds[i + 1]
        idx = order[lo_b:hi_b]
        rr = r_s[lo_b:hi_b] - NODES_PER_CORE * i

        d = np.bincount(rr, minlength=NODES_PER_CORE)
        rem = d & 3
        rows4_n = (d >> 2) + (rem == 3)
        rows2_n = (rem == 2).astype(np.int64)
        rows1_n = (rem == 1).astype(np.int64)
        if (
            rows4_n.sum() > R4_CAP
            or rows2_n.sum() > R2_CAP
            or rows1_n.sum() > R1_CAP
        ):
            # Spill whole tail nodes to a host-side fixup.
            cut = min(
                int(np.searchsorted(np.cumsum(rows4_n), R4_CAP, side="right")),
                int(np.searchsorted(np.cumsum(rows2_n), R2_CAP, side="right")),
                int(np.searchsorted(np.cumsum(rows1_n), R1_CAP, side="right")),
            )
            sp = rr >= cut
            spill_ids.append(idx[sp])
            idx, rr = idx[~sp], rr[~sp]
            d = np.bincount(rr, minlength=NODES_PER_CORE)
            rem = d & 3
            rows4_n = (d >> 2) + (rem == 3)
            rows2_n = (rem == 2).astype(np.int64)
            rows1_n = (rem == 1).astype(np.int64)

        def excl_cumsum(a):
            s = np.zeros_like(a)
            np.cumsum(a[:-1], out=s[1:])
            return s

        rs4 = excl_cumsum(rows4_n)
        rs2 = excl_cumsum(rows2_n)
        rs1 = excl_cumsum(rows1_n)
        node_first = excl_cumsum(d)

        rank = np.arange(len(rr)) - node_first[rr]
        e_rem = rem[rr]
        n_s4_edges = np.where(e_rem == 3, d[rr], (d[rr] >> 2) << 2)
        m4 = rank < n_s4_edges
        m2 = (~m4) & (e_rem == 2)
        m1 = (~m4) & (e_rem == 1)

        tokens = np.zeros((128, C_CHUNKS, 128), np.float16)

        row4 = rs4[rr[m4]] + (rank[m4] >> 2)
        lc = lc4_map[row4]
        p = j4_map[row4] * 4 + (rank[m4] & 3)
        tokens[p, lc, 0:64] = hi_all[idx[m4]]
        tokens[p, lc, 64:128] = lo_all[idx[m4]]

        row2 = rs2[rr[m2]]
        slot2 = rank[m2] - n_s4_edges[m2]
        lc = lc2_map[row2]
        p = j2_map[row2] * 2 + slot2
        tokens[p, lc, 0:64] = hi_all[idx[m2]]
        tokens[p, lc, 64:128] = lo_all[idx[m2]]

        row1 = rs1[rr[m1]]
        lc = lc1_map[row1]
        p = j1_map[row1]
        tokens[p, lc, 0:64] = hi_all[idx[m1]]
        tokens[p, lc, 64:128] = lo_all[idx[m1]]

        in_maps.append(
            {"tokens": tokens, "ones4": ones4, "ones2": ones2, "ones1": ones1}
        )
        meta.append((rows4_n, rs4, rows2_n, rs2, rows1_n, rs1))

    from concourse.bass_utils import run_bass_kernel_spmd

    nc = _build_nc()
    res = run_bass_kernel_spmd(nc, in_maps, core_ids=list(range(N_CORES)))
    LAST_RESULT = res

    # ---- unshard: decode device rows back to row-major order, fold per node.
    # The ROW->(block, partition) placement is unchanged (row & 127 spans the
    # block's partitions in order); only chunk order within a group differs,
    # which the out tensor never sees.
    full = np.zeros((N_NODES, N_FEAT), np.float32)
    for i in range(N_CORES):
        dev = res.results[i]["out"]  # [128, N_BLOCKS, 64]
        rows4_n, rs4, rows2_n, rs2, rows1_n, rs1 = meta[i]
        block = full[i * NODES_PER_CORE : (i + 1) * NODES_PER_CORE]

        arr4 = dev[:, 0:N4_BLOCKS, :].transpose(1, 0, 2).reshape(R4_CAP, 64)
        nz = rows4_n > 0
        if nz.any():
            block[nz] = np.add.reduceat(arr4, rs4[nz])

        arr2 = (
            dev[:, N4_BLOCKS : N4_BLOCKS + N2_BLOCKS, :]
            .transpose(1, 0, 2)
            .reshape(R2_CAP, 64)
        )
        m2n = rows2_n > 0
        if m2n.any():
            block[m2n] += arr2[rs2[m2n]]

        arr1 = (
            dev[:, N4_BLOCKS + N2_BLOCKS :, :].transpose(1, 0, 2).reshape(R1_CAP, 64)
        )
        m1n = rows1_n > 0
        if m1n.any():
            block[m1n] += arr1[rs1[m1n]]

    if spill_ids:
        sp = np.concatenate(spill_ids)
        np.add.at(full, r[sp], edges[sp])

    return full


# revision 15
# speedup vs baseline: 1.2062x; 1.0945x over previous
"""Trainium2 Bass kernel: segment_sum of edge features into nodes (GNN aggregation).

out[n, :] = sum of edges[e, :] over edges with receivers[e] == n, for
n in [0, 100000), edges [1000000, 64] fp32 — distributed over 8 NeuronCores.
Cores are value-sharded by receiver range (12500 nodes each, disjoint), so no
cross-core reduction is needed; the host concatenates the shards.

Device algorithm (degree-slotted static-ones matmul, fused):
  - Host sorts each core's edges by receiver and packs them into "node-rows"
    of 4/2/1 slots (three regions by degree remainder, minimizing padding);
    a chunk = 128 slots.
  - The stationary operand is a STATIC block-ones matrix (e.g. [128, 32] with
    ones[s, j] = 1 iff s//4 == j): out row j = sum of row j's slots. No
    per-chunk weight generation at all (no one-hot; VectorEngine is idle).
  - Edge fp32 values ride as fp16 hi + fp16 lo halves: the matmul output
    access pattern wraps both 64-column halves onto the same PSUM columns and
    PSUM's per-element has_written accumulate folds hi+lo in hardware
    (end-to-end error ~2e-7 relative).
  - One matmul processes up to 4 chunks (rhs free dim 512, 4D wrapped out
    AP), cutting the PE instruction stream ~4x; column tiling
    (tile_position=(0, 32b)) packs 4 chunk-columns per 128-partition PSUM
    block, 8 blocks fill one 2KB PSUM bank exactly; one ScalarEngine copy
    flushes the bank. Input DMAs ride the Sync-engine HWDGE ring, output DMAs
    the Scalar-engine ring so the streams don't serialize on one FIFO.
  - Small S1/S2 groups are interleaved into the S4 stream to prime the DMA
    pipeline and pad bubbles; group sizes ramp down at the tail.
  - Host folds the ~3 rows per node with np.add.reduceat (S4) and vectorized
    adds (S2/S1), then fixes up any capacity-spilled edges.
"""

import os

import numpy as np

N_EDGES = 1_000_000
N_NODES = 100_000
N_FEAT = 64
N_CORES = 8
NODES_PER_CORE = N_NODES // N_CORES  # 12500

# group sizes (in 128-row blocks); shared by host packing and device schedule
G4 = [8] * 29 + [4, 2, 1, 1]  # 240 blocks: rows cap 30720 (mean ~30500)
G2 = [8] * 3  # 24 blocks: cap 3072 (mean ~3125; small spills possible)
G1 = [8] * 3  # 24 blocks: cap 3072
N4_BLOCKS = sum(G4)
N2_BLOCKS = sum(G2)
N1_BLOCKS = sum(G1)
N_BLOCKS = N4_BLOCKS + N2_BLOCKS + N1_BLOCKS  # 288
R4_CAP = N4_BLOCKS * 128
R2_CAP = N2_BLOCKS * 128
R1_CAP = N1_BLOCKS * 128
C4 = N4_BLOCKS * 4
C2 = N2_BLOCKS * 2
C1 = N1_BLOCKS * 1
C_CHUNKS = C4 + C2 + C1  # 1032

_NC_CACHE = None
LAST_RESULT = None


def _region_layout(groups, cpb):
    """Per-group (block_start, chunk_start) offsets within a region."""
    bs, cs, out = 0, 0, []
    for gw in groups:
        out.append((bs, cs))
        bs += gw
        cs += gw * cpb
    return out


_L4 = _region_layout(G4, 4)
_L2 = _region_layout(G2, 2)
_L1 = _region_layout(G1, 1)


def _row_maps():
    """Vectorized row-id -> (chunk index, column j) maps per region.

    Within a group of gw blocks, chunks are laid out b-major (lc = b*gw +
    blkin) so one matmul's rhs spans up to 4 consecutive chunks of the same
    column-group b across blocks.
    """
    maps = {}
    for name, groups, layout, cpb, cbase in (
        ("s4", G4, _L4, 4, 0),
        ("s2", G2, _L2, 2, C4),
        ("s1", G1, _L1, 1, C4 + C2),
    ):
        cap = sum(groups) * 128
        rows = np.arange(cap)
        block = rows >> 7
        gstarts = np.array([b for b, _ in layout])
        g = np.searchsorted(gstarts, block, side="right") - 1
        blkin = block - gstarts[g]
        gw = np.array(groups)[g]
        cstart = cbase + np.array([c for _, c in layout])[g]
        rows_per_col = 128 // cpb  # 32 / 64 / 128
        b = (rows & 127) // rows_per_col
        j = (rows & 127) % rows_per_col
        lc = cstart + b * gw + blkin
        maps[name] = (lc.astype(np.int64), j.astype(np.int64))
    return maps


_ROW_MAPS = _row_maps()


def _build_nc():
    global _NC_CACHE
    if _NC_CACHE is not None:
        return _NC_CACHE

    import concourse.bass as bass
    import concourse.tile as tile
    from concourse import bacc, mybir

    F16 = mybir.dt.float16
    F32 = mybir.dt.float32

    nc = bacc.Bacc("TRN2", target_bir_lowering=False)
    tokens = nc.dram_tensor("tokens", [128, C_CHUNKS, 128], F16, kind="ExternalInput")
    ones4 = nc.dram_tensor("ones4", [128, 32], F16, kind="ExternalInput")
    ones2 = nc.dram_tensor("ones2", [128, 64], F16, kind="ExternalInput")
    ones1 = nc.dram_tensor("ones1", [128, 128], F16, kind="ExternalInput")
    out = nc.dram_tensor("out", [128, N_BLOCKS, 64], F32, kind="ExternalOutput")

    with tile.TileContext(nc) as tc:
        with (
            tc.tile_pool(name="const", bufs=1) as const,
            tc.tile_pool(name="tok", bufs=6) as tokp,
            tc.tile_pool(name="ps", bufs=4, space="PSUM") as psp,
            tc.tile_pool(name="stage", bufs=3) as stp,
        ):
            ones4_t = const.tile([128, 32], F16)
            nc.scalar.dma_start(ones4_t[:], ones4[:])
            ones2_t = const.tile([128, 64], F16)
            nc.scalar.dma_start(ones2_t[:], ones2[:])
            ones1_t = const.tile([128, 128], F16)
            nc.scalar.dma_start(ones1_t[:], ones1[:])

            # (groups, layout, cpb, ones tile, out-rows/chunk, cbase, bbase)
            regs = [
                (G1, _L1, 1, ones1_t, 128, C4 + C2, N4_BLOCKS + N2_BLOCKS),
                (G2, _L2, 2, ones2_t, 64, C4, N4_BLOCKS),
                (G4, _L4, 4, ones4_t, 32, 0, 0),
            ]
            emit = [(0, 0), (1, 0)]
            small = [(0, k) for k in range(1, len(G1))] + [
                (1, k) for k in range(1, len(G2))
            ]
            for k in range(len(G4)):
                emit.append((2, k))
                if k % 3 == 2 and small:
                    emit.append(small.pop(0))
            emit += small

            flush_tick = 0
            for ridx, gidx in emit:
                groups, layout, cpb, ones_t, cols, cbase, bbase = regs[ridx]
                gw = groups[gidx]
                b0, cs = layout[gidx]
                c0 = cbase + cs
                nchunks = gw * cpb
                tok = tokp.tile([128, 8 * 4, 128], F16, tag="tok")
                nc.sync.dma_start(
                    tok[:, 0:nchunks, :], tokens[:, c0 : c0 + nchunks, :])
                ps = psp.tile([128, 8 * 64], F32, tag="ps")
                for b in range(cpb):
                    for blk0 in range(0, gw, 4):
                        w = min(4, gw - blk0)
                        pslice = ps[cols * b : cols * (b + 1),
                                    blk0 * 64 : (blk0 + w) * 64]
                        o = bass.AP(
                            pslice.tensor, pslice.offset,
                            [list(pslice.ap[0]), [64, w], [0, 2], [1, 64]])
                        nc.tensor.matmul(
                            out=o, lhsT=ones_t[:],
                            rhs=tok[:, b * gw + blk0 : b * gw + blk0 + w, :],
                            start=True, stop=True,
                            tile_position=(0, cols * b))
                stage = stp.tile([128, 8 * 64], F32, tag="stage")
                if flush_tick % 2:
                    nc.vector.tensor_copy(stage[:, 0 : gw * 64], ps[:, 0 : gw * 64])
                else:
                    nc.scalar.copy(stage[:, 0 : gw * 64], ps[:, 0 : gw * 64])
                flush_tick += 1
                nc.scalar.dma_start(
                    out[:, bbase + b0 : bbase + b0 + gw, :],
                    stage[:, 0 : gw * 64])
    nc.compile()
    _NC_CACHE = nc
    return nc


def _numpy_segment_sum(edges, receivers):
    out = np.zeros((N_NODES, N_FEAT), np.float32)
    r = np.asarray(receivers).astype(np.int64)
    ok = (r >= 0) & (r < N_NODES)
    np.add.at(out, r[ok], np.asarray(edges, np.float32)[ok])
    return out


def kernel(edges, nodes, receivers):
    global LAST_RESULT

    edges = np.ascontiguousarray(edges, dtype=np.float32)
    n_nodes = nodes.shape[0]
    r = np.asarray(receivers).astype(np.int64)
    if (
        edges.shape != (N_EDGES, N_FEAT)
        or n_nodes != N_NODES
        or r.shape != (N_EDGES,)
        or os.environ.get("KERNEL_FORCE_NUMPY")
    ):
        return _numpy_segment_sum(edges, receivers)

    order = np.argsort(r, kind="stable")
    r_s = r[order]
    bounds = np.searchsorted(r_s, NODES_PER_CORE * np.arange(N_CORES + 1))

    hi_all = edges.astype(np.float16)
    lo_all = (edges - hi_all.astype(np.float32)).astype(np.float16)

    ar = np.arange(128)
    ones4 = np.zeros((128, 32), np.float16)
    ones4[ar, ar // 4] = 1.0
    ones2 = np.zeros((128, 64), np.float16)
    ones2[ar, ar // 2] = 1.0
    ones1 = np.zeros((128, 128), np.float16)
    ones1[ar, ar] = 1.0

    lc4_map, j4_map = _ROW_MAPS["s4"]
    lc2_map, j2_map = _ROW_MAPS["s2"]
    lc1_map, j1_map = _ROW_MAPS["s1"]

    in_maps = []
    spill_ids = []
    meta = []
    for i in range(N_CORES):
        lo_b, hi_b = bounds[i], bounds[i + 1]
        idx = order[lo_b:hi_b]
        rr = r_s[lo_b:hi_b] - NODES_PER_CORE * i

        d = np.bincount(rr, minlength=NODES_PER_CORE)
        rem = d & 3
        rows4_n = (d >> 2) + (rem == 3)
        rows2_n = (rem == 2).astype(np.int64)
        rows1_n = (rem == 1).astype(np.int64)
        if (
            rows4_n.sum() > R4_CAP
            or rows2_n.sum() > R2_CAP
            or rows1_n.sum() > R1_CAP
        ):
            # Spill whole tail nodes to a host-side fixup.
            cut = min(
                int(np.searchsorted(np.cumsum(rows4_n), R4_CAP, side="right")),
                int(np.searchsorted(np.cumsum(rows2_n), R2_CAP, side="right")),
                int(np.searchsorted(np.cumsum(rows1_n), R1_CAP, side="right")),
            )
            sp = rr >= cut
            spill_ids.append(idx[sp])
            idx, rr = idx[~sp], rr[~sp]
            d = np.bincount(rr, minlength=NODES_PER_CORE)
            rem = d & 3
            rows4_n = (d >> 2) + (rem == 3)
            rows2_n = (rem == 2).astype(np.int64)
            rows1_n = (rem == 1).astype(np.int64)

        def excl_cumsum(a):
            s = np.zeros_like(a)
            np.cumsum(a[:-1], out=s[1:])
            return s

        rs4 = excl_cumsum(rows4_n)
        rs2 = excl_cumsum(rows2_n)
        rs1 = excl_cumsum(rows1_n)
        node_first = excl_cumsum(d)

        rank = np.arange(len(rr)) - node_first[rr]
        e_rem = rem[rr]
        n_s4_edges = np.where(e_rem == 3, d[rr], (d[rr] >> 2) << 2)
        m4 = rank < n_s4_edges
        m2 = (~m4) & (e_rem == 2)
        m1 = (~m4) & (e_rem == 1)

        tokens = np.zeros((128, C_CHUNKS, 128), np.float16)

        row4 = rs4[rr[m4]] + (rank[m4] >> 2)
        lc = lc4_map[row4]
        p = j4_map[row4] * 4 + (rank[m4] & 3)
        tokens[p, lc, 0:64] = hi_all[idx[m4]]
        tokens[p, lc, 64:128] = lo_all[idx[m4]]

        row2 = rs2[rr[m2]]
        slot2 = rank[m2] - n_s4_edges[m2]
        lc = lc2_map[row2]
        p = j2_map[row2] * 2 + slot2
        tokens[p, lc, 0:64] = hi_all[idx[m2]]
        tokens[p, lc, 64:128] = lo_all[idx[m2]]

        row1 = rs1[rr[m1]]
        lc = lc1_map[row1]
        p = j1_map[row1]
        tokens[p, lc, 0:64] = hi_all[idx[m1]]
        tokens[p, lc, 64:128] = lo_all[idx[m1]]

        in_maps.append(
            {"tokens": tokens, "ones4": ones4, "ones2": ones2, "ones1": ones1}
        )
        meta.append((rows4_n, rs4, rows2_n, rs2, rows1_n, rs1))

    from concourse.bass_utils import run_bass_kernel_spmd

    nc = _build_nc()
    res = run_bass_kernel_spmd(nc, in_maps, core_ids=list(range(N_CORES)))
    LAST_RESULT = res

    # ---- unshard: decode device rows back to row-major order, fold per node.
    # The ROW->(block, partition) placement is unchanged (row & 127 spans the
    # block's partitions in order); only chunk order within a group differs,
    # which the out tensor never sees.
    full = np.zeros((N_NODES, N_FEAT), np.float32)
    for i in range(N_CORES):
        dev = res.results[i]["out"]  # [128, N_BLOCKS, 64]
        rows4_n, rs4, rows2_n, rs2, rows1_n, rs1 = meta[i]
        block = full[i * NODES_PER_CORE : (i + 1) * NODES_PER_CORE]

        arr4 = dev[:, 0:N4_BLOCKS, :].transpose(1, 0, 2).reshape(R4_CAP, 64)
        nz = rows4_n > 0
        if nz.any():
            block[nz] = np.add.reduceat(arr4, rs4[nz])

        arr2 = (
            dev[:, N4_BLOCKS : N4_BLOCKS + N2_BLOCKS, :]
            .transpose(1, 0, 2)
            .reshape(R2_CAP, 64)
        )
        m2n = rows2_n > 0
        if m2n.any():
            block[m2n] += arr2[rs2[m2n]]

        arr1 = (
            dev[:, N4_BLOCKS + N2_BLOCKS :, :].transpose(1, 0, 2).reshape(R1_CAP, 64)
        )
        m1n = rows1_n > 0
        if m1n.any():
            block[m1n] += arr1[rs1[m1n]]

    if spill_ids:
        sp = np.concatenate(spill_ids)
        np.add.at(full, r[sp], edges[sp])

    return full
